# revision 10
# baseline (speedup 1.0000x reference)
"""Fused NonLocalBlock2D kernel for Trainium2 (8 NeuronCores, batch-parallel).

Per-core computation (one batch sample, C=64, C2=32, N=64*64=4096):
  xf  = x[b]                          [C, N]
  f   = xf^T xf                       [N, N]   (never in HBM)
  p   = softmax(f, axis=-1)
  gx  = g_w xf + g_b                  [N, C2]
  y   = p gx                          [N, C2]
  z   = W_w y^T + W_b + xf            [C, N]

Structure (v2):
  - g_b folds into b_eff = W_w g_b + W_b because softmax rows sum to 1.
  - Numerical shift: e[m,n] = exp(s[m,n] - D[n]) with D = colwise |x|^2;
    per-column constants cancel in num/den.  The shift is applied by a
    DVE pass (psum + negDbc -> sbuf fp32) so the score matmul stays K=64
    (1 cycle/col PE feed; K=65 would cost 2 cycles/col).
  - S-matmul operands are fp16 (both must be <=16-bit together; mixing
    32-bit with 16-bit inputs is rejected by HW).  exp output is bf16
    (range needs 8-bit exponent), gx stationary is fp16.
  - exp runs on grouped [128, 4096] tiles to amortize ACT fixed cost.
  - y0 row 32 = column sums of e (ones column in gx stationary) -> den.
  - 1/den broadcast across partitions via K=1 PE matmul; tail is fp32.
  - Software pipelining: y0 matmuls for group g-1 are interleaved with
    score matmuls of group g; the quarter tail is emitted inside the
    first group of the next quarter to keep PE busy.
"""

import numpy as np

_REPO = "/opt/trn_rl_repo"

C = 64
C2 = 32
N = 4096
MC = 128          # m-chunk width (partition dim of e tiles)
NMC = N // MC     # 32 m-chunks
QW = 1024         # n-quarter width
NQ = N // QW      # 4 quarters
HB = 512          # psum-bank width / matmul max free
G = 4             # m-chunks per exp group
NG = NMC // G     # 8 groups

_CACHE = {}


def _ensure_path():
    import sys
    if _REPO not in sys.path:
        sys.path.insert(0, _REPO)


def _build_nc():
    _ensure_path()
    import concourse.tile as tile
    from concourse import bacc, mybir
    from contextlib import ExitStack

    fp32 = mybir.dt.float32
    fp16 = mybir.dt.float16
    bf16 = mybir.dt.bfloat16
    AF = mybir.ActivationFunctionType
    ALU = mybir.AluOpType

    nc = bacc.Bacc(
        "TRN2",
        target_bir_lowering=False,
        debug=False,
        enable_asserts=True,
        num_devices=8,
    )

    xf_d = nc.dram_tensor("xf", [C, N], fp32, kind="ExternalInput").ap()
    gwT_d = nc.dram_tensor("g_wT", [C, C2], fp32, kind="ExternalInput").ap()
    WwT_d = nc.dram_tensor("W_wT", [C2, C], fp32, kind="ExternalInput").ap()
    beff_d = nc.dram_tensor("b_eff", [C, 1], fp32, kind="ExternalInput").ap()
    out_d = nc.dram_tensor("out", [C, N], fp32, kind="ExternalOutput").ap()

    with tile.TileContext(nc) as tc, ExitStack() as ctx:
        persist = ctx.enter_context(tc.tile_pool(name="persist", bufs=1))
        xfo = persist.tile([C, N], fp32)
        xH = persist.tile([C, N], fp16)
        xsq = persist.tile([C, N], fp32)
        negDrow = persist.tile([1, N], fp32)
        negDbc = persist.tile([MC, N], fp32)
        gxH = persist.tile([MC, 33 * NMC], fp16)
        gwT_s = persist.tile([C, C2], fp32)
        WwT_f = persist.tile([C2, C], fp32)
        beff_s = persist.tile([C, 1], fp32)
        onesMC = persist.tile([1, MC], fp32)
        onesC2 = persist.tile([1, C2], fp32)
        negonf = persist.tile([C, 1], fp32)

        nc.sync.dma_start(xfo[:], xf_d)
        nc.sync.dma_start(gwT_s[:], gwT_d)
        nc.sync.dma_start(WwT_f[:], WwT_d)
        nc.sync.dma_start(beff_s[:], beff_d)
        nc.any.memset(onesMC[:], 1.0)
        nc.any.memset(onesC2[:], 1.0)
        nc.any.memset(negonf[:], -1.0)
        nc.any.memset(gxH[:], 1.0)

        nc.vector.tensor_copy(xH[:], xfo[:])
        nc.gpsimd.tensor_mul(xsq[:], xfo[:], xfo[:])

        s_pool = ctx.enter_context(tc.tile_pool(name="spsum", bufs=3, space="PSUM"))
        y0_pool = ctx.enter_context(tc.tile_pool(name="y0psum", bufs=1, space="PSUM"))
        rbc_pool = ctx.enter_context(tc.tile_pool(name="rbcpsum", bufs=1, space="PSUM"))
        z_pool = ctx.enter_context(tc.tile_pool(name="zpsum", bufs=1, space="PSUM"))

        # negD row then broadcast to 128 partitions (both fp32, exact)
        for j in range(N // HB):
            dp = s_pool.tile([1, HB], fp32, tag="S")
            nc.tensor.matmul(
                dp[:], lhsT=negonf[:], rhs=xsq[:, j * HB : (j + 1) * HB],
                start=True, stop=True,
            )
            nc.vector.tensor_copy(negDrow[:, j * HB : (j + 1) * HB], dp[:])
        for j in range(N // HB):
            db = s_pool.tile([MC, HB], fp32, tag="S")
            nc.tensor.matmul(
                db[:], lhsT=onesMC[:], rhs=negDrow[:, j * HB : (j + 1) * HB],
                start=True, stop=True,
            )
            nc.vector.tensor_copy(negDbc[:, j * HB : (j + 1) * HB], db[:])

        # gx chunks [MC, 32] fp16; 33rd column stays 1.0 from the memset
        for q in range(NMC):
            gp = s_pool.tile([MC, C2], fp32, tag="S")
            nc.tensor.matmul(
                gp[:], lhsT=xfo[:, q * MC : (q + 1) * MC], rhs=gwT_s[:],
                start=True, stop=True,
            )
            nc.vector.tensor_copy(gxH[:, q * 33 : q * 33 + C2], gp[:])

        ssb_pool = ctx.enter_context(tc.tile_pool(name="ssb", bufs=2))
        e_pool = ctx.enter_context(tc.tile_pool(name="e", bufs=2))
        ysb_pool = ctx.enter_context(tc.tile_pool(name="ysb", bufs=2))
        r_pool = ctx.enter_context(tc.tile_pool(name="r", bufs=2))
        y1_pool = ctx.enter_context(tc.tile_pool(name="y1", bufs=2))
        o_pool = ctx.enter_context(tc.tile_pool(name="osb", bufs=2))

        def emit_y(y0, e_t, g, jj):
            q = g * G + jj // 2
            h = jj % 2
            nc.tensor.matmul(
                y0[:, h * HB : (h + 1) * HB],
                lhsT=gxH[:, q * 33 : (q + 1) * 33],
                rhs=e_t[:, jj * HB : (jj + 1) * HB],
                start=(q == 0),
                stop=(q == NMC - 1),
            )

        tail_state = {}

        def tail_pre(nq, y0):
            y_sb = ysb_pool.tile([33, QW], fp32)
            nc.vector.tensor_copy(y_sb[:], y0[:])
            r_row = r_pool.tile([1, QW], fp32)
            nc.vector.reciprocal(r_row[:], y_sb[C2 : C2 + 1, :])
            tail_state[nq] = (y_sb, r_row)

        def tail_post(nq):
            n0 = nq * QW
            y_sb, r_row = tail_state.pop(nq)
            for h in range(2):
                rbc = rbc_pool.tile([C2, HB], fp32)
                nc.tensor.matmul(
                    rbc[:], lhsT=onesC2[:], rhs=r_row[:, h * HB : (h + 1) * HB],
                    start=True, stop=True,
                )
                y1 = y1_pool.tile([C2, HB], fp32)
                nc.vector.tensor_mul(y1[:], y_sb[0:C2, h * HB : (h + 1) * HB], rbc[:])
                z_t = z_pool.tile([C, HB], fp32)
                nc.tensor.matmul(
                    z_t[:], lhsT=WwT_f[:], rhs=y1[:], start=True, stop=True,
                )
                o_t = o_pool.tile([C, HB], fp32)
                nc.vector.scalar_tensor_tensor(
                    o_t[:], z_t[:], beff_s[:],
                    xfo[:, n0 + h * HB : n0 + (h + 1) * HB],
                    op0=ALU.add, op1=ALU.add,
                )
                nc.sync.dma_start(out_d[:, n0 + h * HB : n0 + (h + 1) * HB], o_t[:])

        # Global pipeline over 32 group-slots (8 per quarter), lag-2:
        # slot gg runs score matmuls for its own group, y0 matmuls for
        # group gg-2 (whose exp finished during slot gg-1), and emits the
        # tail of quarter nq-1 in slot g==2 of quarter nq.
        NSLOT = NQ * NG
        y_tiles = {}
        pend = {}  # gg -> (e_t, g, y0)
        for gg in range(NSLOT + 2):
            if gg < NSLOT:
                nq, g = divmod(gg, NG)
                n0 = nq * QW
                if g == 0:
                    y_tiles[nq] = y0_pool.tile([33, QW], fp32, name="y0", tag="y0")
                s_sb = ssb_pool.tile([MC, G * QW], fp32)
                for jj in range(2 * G):
                    q = g * G + jj // 2
                    h = jj % 2
                    c0 = n0 + h * HB
                    s_t = s_pool.tile([MC, HB], fp32, tag="S")
                    nc.tensor.matmul(
                        s_t[:],
                        lhsT=xH[:, q * MC : (q + 1) * MC],
                        rhs=xH[:, c0 : c0 + HB],
                        start=True, stop=True,
                    )
                    nc.vector.scalar_tensor_tensor(
                        s_sb[:, jj * HB : (jj + 1) * HB],
                        s_t[:], 0.0, negDbc[:, c0 : c0 + HB],
                        op0=ALU.add, op1=ALU.add,
                    )
                    if gg - 2 in pend:
                        emit_y(*pend[gg - 2], jj)
                if gg - 2 in pend:
                    del pend[gg - 2]
                if g == 1 and nq > 0:
                    tail_pre(nq - 1, y_tiles.pop(nq - 1))
                if g == 2 and nq > 0:
                    tail_post(nq - 1)
                e_t = e_pool.tile([MC, G * QW], bf16)
                nc.scalar.activation(e_t[:], s_sb[:], AF.Exp)
                pend[gg] = (y_tiles[nq], e_t, g)
            else:
                for jj in range(2 * G):
                    emit_y(*pend[gg - 2], jj)
                del pend[gg - 2]
        tail_pre(NQ - 1, y_tiles.pop(NQ - 1))
        tail_post(NQ - 1)

    nc.compile()
    return nc


def _get_nc():
    if "nc" not in _CACHE:
        _CACHE["nc"] = _build_nc()
    return _CACHE["nc"]


def _run(inputs, trace=False, **kw):
    _ensure_path()
    from concourse.bass_utils import run_bass_kernel_spmd

    nc = _get_nc()
    x = np.ascontiguousarray(np.asarray(inputs["x"], dtype=np.float32))
    g_w = np.asarray(inputs["g_w"], dtype=np.float32)
    g_b = np.asarray(inputs["g_b"], dtype=np.float32)
    W_w = np.asarray(inputs["W_w"], dtype=np.float32)
    W_b = np.asarray(inputs["W_b"], dtype=np.float32)

    gwT = np.ascontiguousarray(g_w.T)                         # [C, C2]
    WwT = np.ascontiguousarray(W_w.T)                         # [C2, C]
    b_eff = (
        W_w.astype(np.float64) @ g_b.astype(np.float64) + W_b.astype(np.float64)
    ).astype(np.float32).reshape(C, 1)

    B = x.shape[0]
    in_maps = [
        {
            "xf": np.ascontiguousarray(x[i].reshape(C, N)),
            "g_wT": gwT,
            "W_wT": WwT,
            "b_eff": b_eff,
        }
        for i in range(B)
    ]
    res = run_bass_kernel_spmd(nc, in_maps, list(range(B)), trace=trace, **kw)
    out = np.stack([res.results[i]["out"].reshape(C, 64, 64) for i in range(B)])
    return res, out.astype(np.float32)


def kernel(**inputs):
    _, out = _run(inputs, trace=False)
    return out


# revision 13
# speedup vs baseline: 1.3406x; 1.3406x over previous
"""Fused NonLocalBlock2D kernel for Trainium2 (8 NeuronCores, batch-parallel).

Per-core computation (one batch sample, C=64, C2=32, N=64*64=4096):
  xf  = x[b]                          [C, N]
  f   = xf^T xf                       [N, N]   (never in HBM)
  p   = softmax(f, axis=-1)
  gx  = g_w xf + g_b                  [N, C2]
  y   = p gx                          [N, C2]
  z   = W_w y^T + W_b + xf            [C, N]

Structure (v2.1):
  - g_b folds into b_eff = W_w g_b + W_b because softmax rows sum to 1.
  - Numerical shift: e[m,n] = exp(s[m,n] - D[n]) with D = colwise |x|^2;
    per-column constants cancel in num/den.  The shift is a DVE pass
    (psum-pair + negDbc -> sbuf fp32, [128,1024] at a time) keeping the
    score matmul at K=64 (1 cycle/col PE feed).
  - No plain-fp32 matmuls anywhere: HW splits those into LOW/HIGH double
    passes at 4 cyc/col.  negD chain runs in fp16 (any rounding of D is a
    per-column factor that cancels), gx and the tail matmuls in f32r.
  - S-matmul operands fp16, exp output bf16 (needs 8-bit exponent), gx
    stationary fp16 with a ones column (row 32 of y0 = den).
  - exp on grouped [128, 4096] tiles; reciprocal of den on ACT
    (AF.Reciprocal); 1/den broadcast across partitions via K=1 f32r
    matmul.
  - Software pipelining with lag 2: y0 matmuls for group g-2 interleave
    with score matmuls of group g; quarter tails split into an ACT/DVE
    prelude (slot g==1) and the PE part (slot g==2).
"""

import numpy as np

_REPO = "/opt/trn_rl_repo"

C = 64
C2 = 32
N = 4096
MC = 128          # m-chunk width (partition dim of e tiles)
NMC = N // MC     # 32 m-chunks
QW = 1024         # n-quarter width
NQ = N // QW      # 4 quarters
HB = 512          # psum-bank width / matmul max free
G = 4             # m-chunks per exp group
NG = NMC // G     # 8 groups

_CACHE = {}


def _ensure_path():
    import sys
    if _REPO not in sys.path:
        sys.path.insert(0, _REPO)


def _build_nc():
    _ensure_path()
    import concourse.tile as tile
    from concourse import bacc, mybir
    from contextlib import ExitStack

    fp32 = mybir.dt.float32
    f32r = mybir.dt.float32r
    fp16 = mybir.dt.float16
    bf16 = mybir.dt.bfloat16
    AF = mybir.ActivationFunctionType
    ALU = mybir.AluOpType

    nc = bacc.Bacc(
        "TRN2",
        target_bir_lowering=False,
        debug=False,
        enable_asserts=True,
        num_devices=8,
    )

    xf_d = nc.dram_tensor("xf", [C, N], fp32, kind="ExternalInput").ap()
    gwT_d = nc.dram_tensor("g_wT", [C, C2], fp32, kind="ExternalInput").ap()
    WwT_d = nc.dram_tensor("W_wT", [C2, C], fp32, kind="ExternalInput").ap()
    beff_d = nc.dram_tensor("b_eff", [C, 1], fp32, kind="ExternalInput").ap()
    out_d = nc.dram_tensor("out", [C, N], fp32, kind="ExternalOutput").ap()

    with tile.TileContext(nc) as tc, ExitStack() as ctx:
        persist = ctx.enter_context(tc.tile_pool(name="persist", bufs=1))
        xfo = persist.tile([C, N], fp32)
        xH = persist.tile([C, N], fp16)
        xR = persist.tile([C, N], f32r)
        xsqH = persist.tile([C, N], fp16)
        negDrowH = persist.tile([1, N], fp16)
        negDbc = persist.tile([MC, N], fp32)
        gxH = persist.tile([MC, 33 * NMC], fp16)
        gwT_s = persist.tile([C, C2], fp32)
        gwTR = persist.tile([C, C2], f32r)
        WwT_f = persist.tile([C2, C], fp32)
        WwT_R = persist.tile([C2, C], f32r)
        beff_s = persist.tile([C, 1], fp32)
        onesMCH = persist.tile([1, MC], fp16)
        onesC2 = persist.tile([1, C2], fp32)
        onesC2R = persist.tile([1, C2], f32r)
        negonfH = persist.tile([C, 1], fp16)

        nc.sync.dma_start(xfo[:], xf_d)
        nc.sync.dma_start(gwT_s[:], gwT_d)
        nc.sync.dma_start(WwT_f[:], WwT_d)
        nc.sync.dma_start(beff_s[:], beff_d)
        nc.any.memset(onesMCH[:], 1.0)
        nc.any.memset(onesC2[:], 1.0)
        nc.any.memset(negonfH[:], -1.0)
        nc.any.memset(gxH[:], 1.0)

        nc.vector.tensor_copy(xH[:], xfo[:])
        nc.scalar.activation(xsqH[:], xfo[:], AF.Square)
        nc.vector.tensor_copy(xR[:], xfo[:])
        nc.vector.tensor_copy(gwTR[:], gwT_s[:])
        nc.vector.tensor_copy(WwT_R[:], WwT_f[:])
        nc.vector.tensor_copy(onesC2R[:], onesC2[:])

        s_pool = ctx.enter_context(tc.tile_pool(name="spsum", bufs=2, space="PSUM"))
        y0_pool = ctx.enter_context(tc.tile_pool(name="y0psum", bufs=1, space="PSUM"))
        rbc_pool = ctx.enter_context(tc.tile_pool(name="rbcpsum", bufs=1, space="PSUM"))
        z_pool = ctx.enter_context(tc.tile_pool(name="zpsum", bufs=1, space="PSUM"))

        # negD row (fp16 matmul; D rounding is per-column and cancels),
        # then broadcast to 128 partitions via K=1 matmul.
        for j in range(N // HB):
            dp = s_pool.tile([1, HB], fp32, tag="S")
            nc.tensor.matmul(
                dp[:], lhsT=negonfH[:], rhs=xsqH[:, j * HB : (j + 1) * HB],
                start=True, stop=True,
            )
            nc.vector.tensor_copy(negDrowH[:, j * HB : (j + 1) * HB], dp[:])
        for j in range(N // HB):
            db = s_pool.tile([MC, HB], fp32, tag="S")
            nc.tensor.matmul(
                db[:], lhsT=onesMCH[:], rhs=negDrowH[:, j * HB : (j + 1) * HB],
                start=True, stop=True,
            )
            nc.scalar.activation(negDbc[:, j * HB : (j + 1) * HB], db[:], AF.Copy)

        # gx chunks [MC, 32] fp16 (f32r matmul); 33rd column stays 1.0
        for q in range(NMC):
            gp = s_pool.tile([MC, C2], fp32, tag="S")
            nc.tensor.matmul(
                gp[:], lhsT=xR[:, q * MC : (q + 1) * MC], rhs=gwTR[:],
                start=True, stop=True,
            )
            nc.vector.tensor_copy(gxH[:, q * 33 : q * 33 + C2], gp[:])

        ssb_pool = ctx.enter_context(tc.tile_pool(name="ssb", bufs=3))
        e_pool = ctx.enter_context(tc.tile_pool(name="e", bufs=3))
        ysb_pool = ctx.enter_context(tc.tile_pool(name="ysb", bufs=2))
        r_pool = ctx.enter_context(tc.tile_pool(name="r", bufs=2))
        y1_pool = ctx.enter_context(tc.tile_pool(name="y1", bufs=2))
        o_pool = ctx.enter_context(tc.tile_pool(name="osb", bufs=2))

        def emit_y(y0, e_t, g, jj):
            q = g * G + jj // 2
            h = jj % 2
            nc.tensor.matmul(
                y0[:, h * HB : (h + 1) * HB],
                lhsT=gxH[:, q * 33 : (q + 1) * 33],
                rhs=e_t[:, jj * HB : (jj + 1) * HB],
                start=(q == 0),
                stop=(q == NMC - 1),
            )

        tail_state = {}

        def tail_pre(nq, y0):
            y_sb = ysb_pool.tile([33, QW], fp32)
            nc.scalar.activation(y_sb[:], y0[:], AF.Copy)
            r_row = r_pool.tile([1, QW], f32r, tag="r_row")
            with nc.allow_low_precision(reason="1/den feeds f32r broadcast matmul"):
                nc.vector.reciprocal(r_row[:], y_sb[C2 : C2 + 1, :])
            tail_state[nq] = (y_sb, r_row)

        def tail_post(nq):
            n0 = nq * QW
            y_sb, r_row = tail_state.pop(nq)
            for h in range(2):
                rbc = rbc_pool.tile([C2, HB], fp32)
                nc.tensor.matmul(
                    rbc[:], lhsT=onesC2R[:], rhs=r_row[:, h * HB : (h + 1) * HB],
                    start=True, stop=True,
                )
                y1 = y1_pool.tile([C2, HB], f32r, tag="y1")
                nc.vector.tensor_mul(y1[:], y_sb[0:C2, h * HB : (h + 1) * HB], rbc[:])
                z_t = z_pool.tile([C, HB], fp32)
                nc.tensor.matmul(
                    z_t[:], lhsT=WwT_R[:], rhs=y1[:], start=True, stop=True,
                )
                o_t = o_pool.tile([C, HB], fp32)
                nc.vector.scalar_tensor_tensor(
                    o_t[:], z_t[:], beff_s[:],
                    xfo[:, n0 + h * HB : n0 + (h + 1) * HB],
                    op0=ALU.add, op1=ALU.add,
                )
                nc.sync.dma_start(out_d[:, n0 + h * HB : n0 + (h + 1) * HB], o_t[:])

        # Global pipeline over 32 group-slots (8 per quarter), lag-2:
        # slot gg runs score matmuls for its own group, y0 matmuls for
        # group gg-2 (whose exp finished during slot gg-1), and emits the
        # tail of quarter nq-1 split across slots g==1 / g==2.
        NSLOT = NQ * NG
        y_tiles = {}
        pend = {}  # gg -> (y0, e_t, g)
        for gg in range(NSLOT + 2):
            if gg < NSLOT:
                nq, g = divmod(gg, NG)
                n0 = nq * QW
                if g == 0:
                    y_tiles[nq] = y0_pool.tile([33, QW], fp32, name="y0", tag="y0")
                s_sb = ssb_pool.tile([MC, G * QW], fp32)
                s_t = None
                for jj in range(2 * G):
                    q = g * G + jj // 2
                    h = jj % 2
                    c0 = n0 + h * HB
                    if h == 0:
                        s_t = s_pool.tile([MC, 2 * HB], fp32, name="s_t", tag="S")
                    nc.tensor.matmul(
                        s_t[:, h * HB : (h + 1) * HB],
                        lhsT=xH[:, q * MC : (q + 1) * MC],
                        rhs=xH[:, c0 : c0 + HB],
                        start=True, stop=True,
                    )
                    if h == 1:
                        t = jj // 2
                        nc.vector.scalar_tensor_tensor(
                            s_sb[:, 2 * t * HB : (2 * t + 2) * HB],
                            s_t[:], 0.0, negDbc[:, n0 : n0 + 2 * HB],
                            op0=ALU.add, op1=ALU.add,
                        )
                    if gg - 2 in pend:
                        emit_y(*pend[gg - 2], jj)
                if gg - 2 in pend:
                    del pend[gg - 2]
                if g == 1 and nq > 0:
                    tail_pre(nq - 1, y_tiles.pop(nq - 1))
                if g == 2 and nq > 0:
                    tail_post(nq - 1)
                e_t = e_pool.tile([MC, G * QW], bf16)
                nc.scalar.activation(e_t[:], s_sb[:], AF.Exp)
                pend[gg] = (y_tiles[nq], e_t, g)
            else:
                for jj in range(2 * G):
                    emit_y(*pend[gg - 2], jj)
                del pend[gg - 2]
        tail_pre(NQ - 1, y_tiles.pop(NQ - 1))
        tail_post(NQ - 1)

    nc.compile()
    return nc


def _get_nc():
    if "nc" not in _CACHE:
        _CACHE["nc"] = _build_nc()
    return _CACHE["nc"]


def _run(inputs, trace=False, **kw):
    _ensure_path()
    from concourse.bass_utils import run_bass_kernel_spmd

    nc = _get_nc()
    x = np.ascontiguousarray(np.asarray(inputs["x"], dtype=np.float32))
    g_w = np.asarray(inputs["g_w"], dtype=np.float32)
    g_b = np.asarray(inputs["g_b"], dtype=np.float32)
    W_w = np.asarray(inputs["W_w"], dtype=np.float32)
    W_b = np.asarray(inputs["W_b"], dtype=np.float32)

    gwT = np.ascontiguousarray(g_w.T)                         # [C, C2]
    WwT = np.ascontiguousarray(W_w.T)                         # [C2, C]
    b_eff = (
        W_w.astype(np.float64) @ g_b.astype(np.float64) + W_b.astype(np.float64)
    ).astype(np.float32).reshape(C, 1)

    B = x.shape[0]
    in_maps = [
        {
            "xf": np.ascontiguousarray(x[i].reshape(C, N)),
            "g_wT": gwT,
            "W_wT": WwT,
            "b_eff": b_eff,
        }
        for i in range(B)
    ]
    res = run_bass_kernel_spmd(nc, in_maps, list(range(B)), trace=trace, **kw)
    out = np.stack([res.results[i]["out"].reshape(C, 64, 64) for i in range(B)])
    return res, out.astype(np.float32)


def kernel(**inputs):
    _, out = _run(inputs, trace=False)
    return out


# revision 14
# speedup vs baseline: 1.5183x; 1.1326x over previous
"""Fused NonLocalBlock2D kernel for Trainium2 (8 NeuronCores, batch-parallel).

Per-core computation (one batch sample, C=64, C2=32, N=64*64=4096):
  xf  = x[b]                          [C, N]
  f   = xf^T xf                       [N, N]   (symmetric, never in HBM)
  p   = softmax(f, axis=-1)
  gx  = g_w xf + g_b                  [N, C2]
  y   = p gx                          [N, C2]
  z   = W_w y^T + W_b + xf            [C, N]

Tricks:
  - g_b folds into b_eff = W_w g_b + W_b because softmax rows sum to 1.
  - Numerical shift: subtract D[n] = sum_c xf[c,n]^2 (the diagonal of f)
    per-column before exp; any per-n constant cancels in y = num/den.
    Realized inside the score matmul with K=65: row 64 of lhsT is ones,
    row 64 of rhs is -D.
  - Row sums d[n] obtained from the same accumulation matmul by adding a
    33rd ones-column to the gx stationary operand (via an extended
    g_wT65 host operand whose row 64 produces an exact 1.0 column).
  - 1/d broadcast across partitions via a K=1 PE matmul with a ones row.
  - All PE operands are float32r (tf32-like 1+8+11): 1 cycle/row when
    the moving free dim >= 512.  HW requires every f32r operand to be
    *written* as f32r by its producer (DVE/ACT convert on writeback);
    fp32 data used by DVE (residual add, reciprocal) is kept in
    separate fp32 tiles.
"""

import numpy as np

_REPO = "/opt/trn_rl_repo"

C = 64
C2 = 32
N = 4096
MC = 128          # m-chunk width (partition dim of E tiles)
NMC = N // MC     # 32 m-chunks
QW = 1024         # n-quarter width (PSUM: 2 banks)
NQ = N // QW      # 4 quarters
HB = 512          # half-quarter / psum-bank width

_CACHE = {}


def _ensure_path():
    import sys
    if _REPO not in sys.path:
        sys.path.insert(0, _REPO)


def _build_nc():
    _ensure_path()
    import concourse.tile as tile
    from concourse import bacc, mybir
    from contextlib import ExitStack

    fp32 = mybir.dt.float32
    f32r = mybir.dt.float32r
    AF = mybir.ActivationFunctionType
    ALU = mybir.AluOpType

    nc = bacc.Bacc(
        "TRN2",
        target_bir_lowering=False,
        debug=False,
        enable_asserts=True,
        num_devices=8,
    )

    xf_d = nc.dram_tensor("xf", [C, N], fp32, kind="ExternalInput").ap()
    gwT_d = nc.dram_tensor("g_wT65", [C + 1, 33], fp32, kind="ExternalInput").ap()
    WwT_d = nc.dram_tensor("W_wT", [C2, C], fp32, kind="ExternalInput").ap()
    beff_d = nc.dram_tensor("b_eff", [C, 1], fp32, kind="ExternalInput").ap()
    out_d = nc.dram_tensor("out", [C, N], fp32, kind="ExternalOutput").ap()

    with tile.TileContext(nc) as tc, ExitStack() as ctx:
        persist = ctx.enter_context(tc.tile_pool(name="persist", bufs=1))
        xfo = persist.tile([C + 1, N], fp32)     # rows 0..63 xf, row 64 = 1.0
        xfoR = persist.tile([C + 1, N], f32r)    # f32r copy (S-mm stationary)
        xfdR = persist.tile([C + 1, N], f32r)    # rows 0..63 xf, row 64 = -D
        xsqR = persist.tile([C, N], f32r)
        gxR = persist.tile([MC, 33 * NMC], f32r)
        gwT_s = persist.tile([C + 1, 33], fp32)
        WwT_f = persist.tile([C2, C], fp32)
        WwT_R = persist.tile([C2, C], f32r)
        beff_s = persist.tile([C, 1], fp32)
        ones1f = persist.tile([1, C2], fp32)
        ones1R = persist.tile([1, C2], f32r)
        negonf = persist.tile([C, 1], fp32)
        negonR = persist.tile([C, 1], f32r)

        nc.sync.dma_start(xfo[0:C, :], xf_d)
        nc.sync.dma_start(gwT_s[:], gwT_d)
        nc.sync.dma_start(WwT_f[:], WwT_d)
        nc.sync.dma_start(beff_s[:], beff_d)
        nc.any.memset(xfo[C : C + 1, :], 1.0)
        nc.any.memset(ones1f[:], 1.0)
        nc.any.memset(negonf[:], -1.0)

        nc.vector.tensor_copy(ones1R[:], ones1f[:])
        nc.vector.tensor_copy(negonR[:], negonf[:])
        nc.vector.tensor_copy(WwT_R[:], WwT_f[:])
        nc.scalar.activation(xfoR[:], xfo[:], AF.Copy)
        nc.vector.tensor_copy(xfdR[0:C, :], xfo[0:C, :])
        nc.gpsimd.tensor_mul(xsqR[:], xfo[0:C, :], xfo[0:C, :])

        s_pool = ctx.enter_context(tc.tile_pool(name="spsum", bufs=2, space="PSUM"))
        y0_pool = ctx.enter_context(tc.tile_pool(name="y0psum", bufs=1, space="PSUM"))
        rbc_pool = ctx.enter_context(tc.tile_pool(name="rbcpsum", bufs=1, space="PSUM"))
        z_pool = ctx.enter_context(tc.tile_pool(name="zpsum", bufs=1, space="PSUM"))

        # -D[n] into xfdR row 64
        for j in range(N // HB):
            dp = s_pool.tile([1, HB], fp32, tag="S")
            nc.tensor.matmul(
                dp[:],
                lhsT=negonR[:],
                rhs=xsqR[:, j * HB : (j + 1) * HB],
                start=True,
                stop=True,
            )
            nc.vector.tensor_copy(xfdR[C : C + 1, j * HB : (j + 1) * HB], dp[:])

        # gx (33rd column == 1.0 via g_wT65 row 64), plain fp32 matmul
        for q in range(NMC):
            gp = s_pool.tile([MC, 33], fp32, tag="S")
            nc.tensor.matmul(
                gp[:],
                lhsT=xfo[:, q * MC : (q + 1) * MC],
                rhs=gwT_s[:],
                start=True,
                stop=True,
            )
            nc.vector.tensor_copy(gxR[:, q * 33 : (q + 1) * 33], gp[:])

        e_pool = ctx.enter_context(tc.tile_pool(name="e", bufs=2))
        ysb_pool = ctx.enter_context(tc.tile_pool(name="ysb", bufs=2))
        r_pool = ctx.enter_context(tc.tile_pool(name="r", bufs=2))
        y1_pool = ctx.enter_context(tc.tile_pool(name="y1", bufs=2))
        o_pool = ctx.enter_context(tc.tile_pool(name="osb", bufs=2))

        for nq in range(NQ):
            n0 = nq * QW
            y0 = y0_pool.tile([33, QW], fp32)
            for q in range(NMC):
                s_t = s_pool.tile([MC, QW], fp32, tag="S")
                for h in range(2):
                    nc.tensor.matmul(
                        s_t[:, h * HB : (h + 1) * HB],
                        lhsT=xfoR[:, q * MC : (q + 1) * MC],
                        rhs=xfdR[:, n0 + h * HB : n0 + (h + 1) * HB],
                        start=True,
                        stop=True,
                    )
                e_t = e_pool.tile([MC, QW], f32r)
                nc.scalar.activation(e_t[:], s_t[:], AF.Exp)
                for h in range(2):
                    nc.tensor.matmul(
                        y0[:, h * HB : (h + 1) * HB],
                        lhsT=gxR[:, q * 33 : (q + 1) * 33],
                        rhs=e_t[:, h * HB : (h + 1) * HB],
                        start=(q == 0),
                        stop=(q == NMC - 1),
                    )

            y_sb = ysb_pool.tile([33, QW], fp32)
            nc.vector.tensor_copy(y_sb[:], y0[:])
            r_row = r_pool.tile([1, QW], f32r)
            with nc.allow_low_precision(reason="1/d feeds f32r broadcast matmul"):
                nc.vector.reciprocal(r_row[:], y_sb[C2 : C2 + 1, :])
            for h in range(2):
                rbc = rbc_pool.tile([C2, HB], fp32)
                nc.tensor.matmul(
                    rbc[:],
                    lhsT=ones1R[:],
                    rhs=r_row[:, h * HB : (h + 1) * HB],
                    start=True,
                    stop=True,
                )
                y1 = y1_pool.tile([C2, HB], f32r)
                nc.vector.tensor_mul(y1[:], y_sb[0:C2, h * HB : (h + 1) * HB], rbc[:])
                z_t = z_pool.tile([C, HB], fp32)
                nc.tensor.matmul(
                    z_t[:],
                    lhsT=WwT_R[:],
                    rhs=y1[:],
                    start=True,
                    stop=True,
                )
                o_t = o_pool.tile([C, HB], fp32)
                nc.vector.scalar_tensor_tensor(
                    o_t[:],
                    z_t[:],
                    beff_s[:],
                    xfo[0:C, n0 + h * HB : n0 + (h + 1) * HB],
                    op0=ALU.add,
                    op1=ALU.add,
                )
                nc.sync.dma_start(out_d[:, n0 + h * HB : n0 + (h + 1) * HB], o_t[:])

    nc.compile()
    return nc


def _get_nc():
    if "nc" not in _CACHE:
        _CACHE["nc"] = _build_nc()
    return _CACHE["nc"]


def _run(inputs, trace=False, **kw):
    _ensure_path()
    from concourse.bass_utils import run_bass_kernel_spmd

    nc = _get_nc()
    x = np.ascontiguousarray(np.asarray(inputs["x"], dtype=np.float32))
    g_w = np.asarray(inputs["g_w"], dtype=np.float32)
    g_b = np.asarray(inputs["g_b"], dtype=np.float32)
    W_w = np.asarray(inputs["W_w"], dtype=np.float32)
    W_b = np.asarray(inputs["W_b"], dtype=np.float32)

    gwT65 = np.zeros((C + 1, 33), dtype=np.float32)
    gwT65[0:C, 0:C2] = g_w.T
    gwT65[C, C2] = 1.0
    WwT = np.ascontiguousarray(W_w.T)                         # [C2, C]
    b_eff = (
        W_w.astype(np.float64) @ g_b.astype(np.float64) + W_b.astype(np.float64)
    ).astype(np.float32).reshape(C, 1)

    B = x.shape[0]
    in_maps = [
        {
            "xf": np.ascontiguousarray(x[i].reshape(C, N)),
            "g_wT65": gwT65,
            "W_wT": WwT,
            "b_eff": b_eff,
        }
        for i in range(B)
    ]
    res = run_bass_kernel_spmd(nc, in_maps, list(range(B)), trace=trace, **kw)
    out = np.stack([res.results[i]["out"].reshape(C, 64, 64) for i in range(B)])
    return res, out.astype(np.float32)


def kernel(**inputs):
    _, out = _run(inputs, trace=False)
    return out
